# revision 1
# baseline (speedup 1.0000x reference)
"""Trainium2 Bass kernel for nn_MemoryNetwork (GRU-style memory network scan).

Model (per reference):
  t_enc = cos(arange(T) * freq + phase)                    [T, D]
  s0 = mean_t(x)                                           [B*C, D]
  tr = arange(T) * mask; x_seq = x[tr]; te_seq = t_enc[tr]
  per step t:
    msg = gelu([x_t, s, te_t] @ msg_W.T + msg_b)
    gi = msg @ W_ih.T + b_ih ; gh = s @ W_hh.T + b_hh
    r = sigmoid(i_r + h_r); z = sigmoid(i_z + h_z)
    n = tanh(i_n + r * h_n)
    s' = (1 - z) * n + z * s
  output: states [T, B, C, D]

Strategy: data-parallel over B*C = 4096 rows -> 8 cores x 512 rows.
On-device layout is feature-major ([D, rows]); matmuls contract over the
partition dim. The 512 rows per core are split into NB=4 independent
blocks whose per-step dependency chains interleave, shrinking the serial
chain's per-op durations (the scan is latency-bound, not
throughput-bound).

One ACT table set (gelu_and_others = {Gelu, Tanh}); sigmoid is exact via
sigma(a) = (1 + tanh(a/2))/2, with 0.5 factors folded into weights:
  hz = tanh(-a_z/2), hr = tanh(+a_r/2)   (one ACT op; z top, r bottom)
  q  = (hr + 1) * hh        with hh = 0.5*(h_n + b_hn) -> q = r*(h_n+b_hn)
  w  = i_n + q              (PE identity-matmul accumulate into PSUM)
  nbar = tanh(-w - b_in) = -n
  d  = s + nbar = s - n
  u2 = (hz + 1) * d         (= 2*(1-z)*(s-n))
  s' = -0.5*u2 + s          (= z*s + (1-z)*n)
Elementwise tensors bf16; PSUM accumulation f32; outputs staged as f32
and DMA'd out every CH steps. The final [D, rows] -> [rows, D] transpose
happens on the host.
"""

import sys

import numpy as np

sys.path.insert(0, "/opt/trn_rl_repo")

import ml_dtypes  # noqa: E402

BF16 = ml_dtypes.bfloat16

T, B, C, D = 256, 64, 64, 64
NCORES = 8
ROWS = (B * C) // NCORES  # 512 rows per core
CH = 8  # timesteps per DMA chunk
NB = 4  # row blocks per core (pipelined independent chains)
BSIZES = [ROWS // NB] * NB
BOFF = [i * (ROWS // NB) for i in range(NB + 1)]

_PROGRAM_CACHE = {}


def _build_program():
    import concourse.bacc as bacc
    import concourse.tile as tile
    from concourse import mybir
    from contextlib import ExitStack

    BF = mybir.dt.bfloat16
    F32 = mybir.dt.float32
    AF = mybir.ActivationFunctionType
    OP = mybir.AluOpType

    # Bacc (not plain Bass): its compile() pass legalizes multi-semaphore
    # waits into event semaphores; raw Bass BIR trips walrus'
    # "Too many sync wait commands" on any instruction joining two streams.
    nc = bacc.Bacc(None, target_bir_lowering=False, debug=False)

    xT = nc.dram_tensor("xT", [T, D, ROWS], BF, kind="ExternalInput")
    s0 = nc.dram_tensor("s0", [D, ROWS], BF, kind="ExternalInput")
    tb = nc.dram_tensor("tb", [1, T, D], BF, kind="ExternalInput")
    # bf16 weights packed column-wise into one [D, 576] blob:
    #   wx [0:64], ws [64:128], wirz [128:256] (z cols first, then r),
    #   whrz [256:384], win [384:448], whn(0.5x) [448:512], prefh row0
    #   [512:576] (0.5*b_hn)
    wblob = nc.dram_tensor("wblob", [D, 9 * D], BF, kind="ExternalInput")
    # identity for the PE w-accumulate, at partitions 64:128
    iblob = nc.dram_tensor("iblob", [2 * D, D], BF, kind="ExternalInput")
    # f32 per-partition vectors [2D, 3]: col0 hrz scale (-0.5 | +0.5),
    # col1 hrz bias (-0.5*b_z | +0.5*b_r), col2 rows 0:64 = -b_in
    fblob = nc.dram_tensor("fblob", [2 * D, 3], F32, kind="ExternalInput")
    outT = nc.dram_tensor("outT", [T, D, ROWS], F32, kind="ExternalOutput")

    with ExitStack() as ctx:
        tc = ctx.enter_context(tile.TileContext(nc))
        consts = ctx.enter_context(tc.tile_pool(name="consts", bufs=1))
        xpool = ctx.enter_context(tc.tile_pool(name="xc", bufs=2))
        opool = ctx.enter_context(tc.tile_pool(name="ostage", bufs=2))
        spool = ctx.enter_context(tc.tile_pool(name="state", bufs=3))
        upool = ctx.enter_context(tc.tile_pool(name="u", bufs=2))
        gpool = ctx.enter_context(tc.tile_pool(name="gates", bufs=2))
        psum = ctx.enter_context(tc.tile_pool(name="psum", bufs=1, space="PSUM"))

        wblob_sb = consts.tile([D, 9 * D], BF, tag="wblob")
        nc.sync.dma_start(out=wblob_sb, in_=wblob[:, :])
        iblob_sb = consts.tile([2 * D, D], BF, tag="iblob")
        nc.sync.dma_start(out=iblob_sb, in_=iblob[:, :])
        fblob_sb = consts.tile([2 * D, 3], F32, tag="fblob")
        nc.sync.dma_start(out=fblob_sb, in_=fblob[:, :])
        tb_sb = consts.tile([1, T, D], BF, tag="tb")
        nc.sync.dma_start(out=tb_sb, in_=tb[:, :, :])

        wx_sb = wblob_sb[:, 0:D]
        ws_sb = wblob_sb[:, D : 2 * D]
        wirz_sb = wblob_sb[:, 2 * D : 4 * D]
        whrz_sb = wblob_sb[:, 4 * D : 6 * D]
        win_sb = wblob_sb[:, 6 * D : 7 * D]
        whn_sb = wblob_sb[:, 7 * D : 8 * D]
        prefh_sb = wblob_sb[0:1, 8 * D : 9 * D]
        ident_sb = iblob_sb[D : 2 * D, :]
        hrz_scale = fblob_sb[:, 0:1]
        hrz_bias = fblob_sb[:, 1:2]
        thbias_sb = fblob_sb[0:D, 2:3]
        ones_sb = consts.tile([1, ROWS], BF)
        nc.vector.memset(ones_sb, 1.0)

        # ACT allows few sync-waits; make the ACT engine observe the fblob
        # DMA lane once so per-step activations only need their PE/DVE wait.
        scratch = consts.tile([2 * D, 3], F32, tag="scratch")
        nc.scalar.copy(out=scratch, in_=fblob_sb)

        s_cur = []
        for b in range(NB):
            st = spool.tile([D, BSIZES[b]], BF, tag=f"state{b}")
            nc.sync.dma_start(out=st, in_=s0[:, BOFF[b] : BOFF[b + 1]])
            s_cur.append(st)

        xc = None
        ostage = None
        for t in range(T):
            k = t % CH
            if k == 0:
                xc = xpool.tile([D, CH, ROWS], BF, tag="xc")
                nc.sync.dma_start(
                    out=xc, in_=xT[t : t + CH, :, :].rearrange("c p r -> p c r")
                )
                ostage = opool.tile([D, CH, ROWS], F32, tag="ostage")

            for b in range(NB):
                rs = slice(BOFF[b], BOFF[b + 1])
                FDB = BSIZES[b]
                s_b = s_cur[b]

                # pmn bank: [0:D] holds the msg pre-activation, which gelu
                # consumes, then i_n overwrites it (start=True); [D:2D] holds
                # hh = 0.5*(h_n + b_hn). One PSUM bank per block for all of it.
                pmn = psum.tile([2 * D, FDB], F32, tag=f"pmn{b}")
                pm = pmn[0:D, :]
                nc.tensor.matmul(
                    pm, tb_sb[:, t, :], ones_sb[:, 0:FDB], start=True, stop=False
                )
                nc.tensor.matmul(pm, wx_sb, xc[:, k, rs], start=False, stop=False)
                nc.tensor.matmul(pm, ws_sb, s_b, start=False, stop=True)

                # u = gelu(pm)
                u = upool.tile([D, FDB], BF, tag=f"u{b}")
                nc.scalar.activation(u, pm, AF.Gelu)

                # rz gates pre-activation (z cols first, then r)
                prz = psum.tile([2 * D, FDB], F32, tag=f"prz{b}")
                nc.tensor.matmul(prz, wirz_sb, u, start=True, stop=False)
                nc.tensor.matmul(prz, whrz_sb, s_b, start=False, stop=True)

                pn = pmn
                nc.tensor.matmul(
                    pn[D : 2 * D, :],
                    prefh_sb,
                    ones_sb[:, 0:FDB],
                    start=True,
                    stop=False,
                )
                nc.tensor.matmul(pn[D : 2 * D, :], whn_sb, s_b, start=False, stop=True)
                # i_n overwrites the consumed msg region (start=True)
                nc.tensor.matmul(pn[0:D, :], win_sb, u, start=True, stop=False)

                # [hz; hr] = tanh(+-0.5 * a + b~)  (z top, r bottom)
                hrz = gpool.tile([2 * D, FDB], BF, tag=f"hrz{b}")
                nc.scalar.activation(
                    hrz, prz, AF.Tanh, bias=hrz_bias, scale=hrz_scale
                )

                # q = (hr + 1) * hh   (all at base partition 64)
                qt = gpool.tile([2 * D, FDB], BF, tag=f"q{b}")
                q = qt[D : 2 * D, :]
                nc.vector.scalar_tensor_tensor(
                    q, hrz[D : 2 * D, :], 1.0, pn[D : 2 * D, :], OP.add, OP.mult
                )
                # w = i_n + q via PE identity accumulate
                nc.tensor.matmul(pn[0:D, :], ident_sb, q, start=False, stop=True)
                # nbar = tanh(-w - b_in) = -n
                nbar = gpool.tile([D, FDB], BF, tag=f"nbar{b}")
                nc.scalar.activation(
                    nbar, pn[0:D, :], AF.Tanh, bias=thbias_sb, scale=-1.0
                )
                # d = s + nbar = s - n
                d = gpool.tile([D, FDB], BF, tag=f"d{b}")
                nc.vector.tensor_add(d, s_b, nbar)
                # u2 = (hz + 1) * d
                u2 = gpool.tile([D, FDB], BF, tag=f"u2{b}")
                nc.vector.scalar_tensor_tensor(
                    u2, hrz[0:D, :], 1.0, d, OP.add, OP.mult
                )
                # s' = -0.5*u2 + s
                s_nxt = spool.tile([D, FDB], BF, tag=f"state{b}")
                nc.vector.scalar_tensor_tensor(
                    s_nxt, u2, -0.5, s_b, OP.mult, OP.add
                )
                # stage output (bf16 -> f32 upcast) off the ACT/DVE engines
                nc.gpsimd.tensor_copy(out=ostage[:, k, rs], in_=s_nxt)
                s_cur[b] = s_nxt

            if k == CH - 1:
                nc.sync.dma_start(
                    out=outT[t - CH + 1 : t + 1, :, :].rearrange("c p r -> p c r"),
                    in_=ostage,
                )

    nc.compile()
    return nc


def _prep_host(x, mask, msg_W, msg_b, W_ih, W_hh, b_ih, b_hh, basis_freq, phase):
    """Host-side prep: sharding/layout + tiny weight preprocessing."""
    x = np.asarray(x, dtype=np.float32)
    mask = np.asarray(mask)
    msg_W = np.asarray(msg_W, np.float32)
    msg_b = np.asarray(msg_b, np.float32)
    W_ih = np.asarray(W_ih, np.float32)
    W_hh = np.asarray(W_hh, np.float32)
    b_ih = np.asarray(b_ih, np.float32)
    b_hh = np.asarray(b_hh, np.float32)
    basis_freq = np.asarray(basis_freq, np.float32)
    phase = np.asarray(phase, np.float32)

    tr = np.arange(T, dtype=np.int64) * mask.astype(np.int64)
    identity_gather = bool(np.array_equal(tr, np.arange(T)))

    xf = x.reshape(T, B * C, D)
    s0_rows = xf.mean(axis=0)  # [B*C, D] f32 (from ungathered x)
    if not identity_gather:
        xf = xf[tr]

    x4 = xf.reshape(T, NCORES, ROWS, D)
    xT8 = [
        np.ascontiguousarray(x4[:, c].transpose(0, 2, 1)).astype(BF16)
        for c in range(NCORES)
    ]
    s08 = [
        np.ascontiguousarray(s0_rows[c * ROWS : (c + 1) * ROWS].T).astype(BF16)
        for c in range(NCORES)
    ]

    ts_ = np.arange(T, dtype=np.float32)[tr]
    te = np.cos(ts_[:, None] * basis_freq[None, :] + phase[None, :])  # [T, D]
    Wt = msg_W[:, 2 * D : 3 * D]
    tb_host = (te @ Wt.T + msg_b[None, :]).astype(BF16).reshape(1, T, D)

    wblob = np.zeros((D, 9 * D), np.float32)
    wblob[:, 0:D] = msg_W[:, 0:D].T
    wblob[:, D : 2 * D] = msg_W[:, D : 2 * D].T
    # z gate columns first, then r (matches hz-top/hr-bottom ACT layout)
    wblob[:, 2 * D : 3 * D] = W_ih[D : 2 * D].T
    wblob[:, 3 * D : 4 * D] = W_ih[0:D].T
    wblob[:, 4 * D : 5 * D] = W_hh[D : 2 * D].T
    wblob[:, 5 * D : 6 * D] = W_hh[0:D].T
    wblob[:, 6 * D : 7 * D] = W_ih[2 * D : 3 * D].T
    wblob[:, 7 * D : 8 * D] = 0.5 * W_hh[2 * D : 3 * D].T
    wblob[0, 8 * D : 9 * D] = 0.5 * b_hh[2 * D : 3 * D]

    iblob = np.zeros((2 * D, D), np.float32)
    iblob[D : 2 * D, :] = np.eye(D, dtype=np.float32)

    fblob = np.zeros((2 * D, 3), np.float32)
    fblob[0:D, 0] = -0.5
    fblob[D : 2 * D, 0] = 0.5
    fblob[0:D, 1] = -0.5 * (b_ih[D : 2 * D] + b_hh[D : 2 * D])
    fblob[D : 2 * D, 1] = 0.5 * (b_ih[0:D] + b_hh[0:D])
    fblob[0:D, 2] = -b_ih[2 * D : 3 * D]

    shared = {
        "tb": tb_host,
        "wblob": wblob.astype(BF16),
        "iblob": iblob.astype(BF16),
        "fblob": fblob,
    }
    in_maps = []
    for c in range(NCORES):
        m = dict(shared)
        m["xT"] = xT8[c]
        m["s0"] = s08[c]
        in_maps.append(m)
    return in_maps


def kernel(**inputs):
    from concourse.bass_utils import run_bass_kernel_spmd

    in_maps = _prep_host(**inputs)

    if "prog" not in _PROGRAM_CACHE:
        _PROGRAM_CACHE["prog"] = _build_program()
    nc = _PROGRAM_CACHE["prog"]

    res = run_bass_kernel_spmd(nc, in_maps, core_ids=list(range(NCORES)))
    _PROGRAM_CACHE["last_results"] = res

    out = np.empty((T, B * C, D), dtype=np.float32)
    for c in range(NCORES):
        outT_c = res.results[c]["outT"]  # [T, D, ROWS] f32
        out[:, c * ROWS : (c + 1) * ROWS, :] = outT_c.transpose(0, 2, 1)
    return out.reshape(T, B, C, D)



# revision 5
# speedup vs baseline: 1.0032x; 1.0032x over previous
"""Trainium2 Bass kernel for nn_MemoryNetwork (GRU-style memory network scan).

Model (per reference):
  t_enc = cos(arange(T) * freq + phase)                    [T, D]
  s0 = mean_t(x)                                           [B*C, D]
  tr = arange(T) * mask; x_seq = x[tr]; te_seq = t_enc[tr]
  per step t:
    msg = gelu([x_t, s, te_t] @ msg_W.T + msg_b)
    gi = msg @ W_ih.T + b_ih ; gh = s @ W_hh.T + b_hh
    r = sigmoid(i_r + h_r); z = sigmoid(i_z + h_z)
    n = tanh(i_n + r * h_n)
    s' = (1 - z) * n + z * s
  output: states [T, B, C, D]

Strategy: data-parallel over B*C = 4096 rows -> 8 cores x 512 rows.
Feature-major layout ([D, rows]); the 512 rows split into 2 staggered
chains of 256 so engine queues overlap across the serial scan.

Cost-model-driven structure (engine busy is dominated by per-instruction
fixed costs, so few wide instructions beat many narrow ones):
  - One chunk tile [128, CH+1, ROWS] holds x (partitions 0:64) stacked
    over the recurrent state s (64:128). The msg matmul contracts K=128
    over [x; s] in a single matmul; gelu then overwrites the consumed
    x slot with u = gelu(msg), so the gate matmuls contract K=128 over
    [u; s] from the same region. State updates write s' into the next
    free slot, which doubles as the output staging buffer: one bf16 DMA
    per chunk ships slots 1..CH straight from the state plane.
  - Time encoding (te @ Wt.T + msg_b) is folded into the gelu bias
    (per-partition bias AP), not a matmul.
  - Sigmoid via tanh: hz = tanh(-(az+bz)/2) = 1-2z, hr = tanh(+(ar+br)/2)
    = 2r-1, computed in one [128, N] activation; then
      q  = (hr+1) * hh       with hh = 0.5*(W_hn@s + b_hn)   [DVE]
      w  = i_n + q           (PE identity-matmul accumulate)
      nbar = tanh(-w - b_in) = -n                            [ACT]
      v  = -0.5*(hz+1)       [Pool tensor_scalar, off critical path]
      d  = s + nbar          [DVE tensor_tensor, 2x mode]
      e  = v * d             [Pool tensor_tensor]
      s' = s + e             [Pool tensor_tensor]
    (GPSIMD cannot access PSUM and rejects TensorScalarPtr, so the Pool
    ops are all plain tensor_tensor/tensor_scalar on SBUF bf16.)
  - Elementwise ops are split DVE/Pool to balance queues (Pool tensor
    ops have no SBUF/PSUM access bubble in the cost model).
Everything elementwise is bf16; PSUM f32; output bf16 (upcast on host).
"""

import sys

import numpy as np

sys.path.insert(0, "/opt/trn_rl_repo")

import ml_dtypes  # noqa: E402

BF16 = ml_dtypes.bfloat16

T, B, C, D = 256, 64, 64, 64
NCORES = 8
ROWS = (B * C) // NCORES  # 512 rows per core
CH = 16  # timesteps per DMA chunk
NCHUNK = T // CH
HB = ROWS // 2  # 256 rows per chain

_PROGRAM_CACHE = {}


def _build_program():
    import concourse.bacc as bacc
    import concourse.tile as tile
    from concourse import mybir
    from contextlib import ExitStack

    BF = mybir.dt.bfloat16
    F32 = mybir.dt.float32
    AF = mybir.ActivationFunctionType
    OP = mybir.AluOpType

    nc = bacc.Bacc(None, target_bir_lowering=False, debug=False)

    xT = nc.dram_tensor("xT", [T, D, ROWS], BF, kind="ExternalInput")
    s0 = nc.dram_tensor("s0", [D, ROWS], BF, kind="ExternalInput")
    # bf16 weights [128, 7*64]:
    #   pmW  [:, 0:64]    = [Wx.T ; Ws.T]           (K=128 -> M=64 msg)
    #   g1W  [:, 64:192]  = [[Wir.T Wiz.T];[Whr.T Whz.T]]  (K=128 -> M=128,
    #        r gates -> out partitions 0:64, z -> 64:128)
    #   g2W  [:, 192:320] = [[0 Win.T];[0.5*Whn.T 0]]      (hh -> 0:64,
    #        i_n -> 64:128; partition bases chosen so every elementwise
    #        op has all operands on one partition range)
    #   idnt [:, 320:384] rows 0:64 = eye(64)
    #   prefh row 0 of [:, 384:448] = 0.5*b_hn
    wblob = nc.dram_tensor("wblob", [2 * D, 7 * D], BF, kind="ExternalInput")
    # f32 consts [128, 259]: cols 0:256 rows 0:64 = tb (te@Wt.T + msg_b).T,
    # col 256 = hzr scale (+0.5 | -0.5), col 257 = hzr bias
    # (+0.5*(b_ir+b_hr) | -0.5*(b_iz+b_hz)), col 258 rows 64:128 = -b_in
    cblob = nc.dram_tensor("cblob", [2 * D, T + 3], F32, kind="ExternalInput")
    outT = nc.dram_tensor("outT", [T, D, ROWS], BF, kind="ExternalOutput")

    with ExitStack() as ctx:
        tc = ctx.enter_context(tile.TileContext(nc))
        consts = ctx.enter_context(tc.tile_pool(name="consts", bufs=1))
        xpool = ctx.enter_context(tc.tile_pool(name="xs", bufs=2))
        gpool = ctx.enter_context(tc.tile_pool(name="gates", bufs=2))
        psum = ctx.enter_context(tc.tile_pool(name="psum", bufs=1, space="PSUM"))

        wblob_sb = consts.tile([2 * D, 7 * D], BF, tag="wblob")
        nc.sync.dma_start(out=wblob_sb, in_=wblob[:, :])
        cblob_sb = consts.tile([2 * D, T + 3], F32, tag="cblob")
        nc.sync.dma_start(out=cblob_sb, in_=cblob[:, :])

        pmW = wblob_sb[:, 0:D]
        g1W = wblob_sb[:, D : 3 * D]
        g2W = wblob_sb[:, 3 * D : 5 * D]
        ident = wblob_sb[0:D, 5 * D : 6 * D]
        prefh = wblob_sb[0:1, 6 * D : 7 * D]
        tb = cblob_sb[0:D, 0:T]
        hzr_scale = cblob_sb[:, T : T + 1]
        hzr_bias = cblob_sb[:, T + 1 : T + 2]
        nb_bias = cblob_sb[D : 2 * D, T + 2 : T + 3]

        ones_sb = consts.tile([1, HB], BF)
        nc.vector.memset(ones_sb, 1.0)

        # let ACT observe the const DMA lane once so steady-state
        # activations only wait on their producer engines
        scratch = consts.tile([2 * D, 4], F32, tag="scratch")
        nc.scalar.copy(out=scratch, in_=cblob_sb[:, T - 1 : T + 3])

        CHS = [slice(0, HB), slice(HB, ROWS)]

        # chunk 0 tile: x into partitions 0:64 slots 0..CH-1, s0 into
        # the state plane (partitions 64:128) slot 0
        xs = xpool.tile([2 * D, CH + 1, ROWS], BF, tag="xs")
        nc.sync.dma_start(
            out=xs[0:D, 0:CH, :], in_=xT[0:CH, :, :].rearrange("c p r -> p c r")
        )
        nc.sync.dma_start(out=xs[D : 2 * D, 0, :], in_=s0[:, :])
        xs_next = None

        for t in range(T):
            k = t % CH
            if k == 1 and t + CH < T:
                xs_next = xpool.tile([2 * D, CH + 1, ROWS], BF, tag="xs")
                nc.sync.dma_start(
                    out=xs_next[0:D, 0:CH, :],
                    in_=xT[t - 1 + CH : t - 1 + 2 * CH, :, :].rearrange(
                        "c p r -> p c r"
                    ),
                )

            hzr_t, q_t, g2_t, v_t = [], [], [], []
            # front phase: msg -> gelu -> gates matmuls -> hzr -> q
            for h in range(2):
                ch = CHS[h]
                pm = psum.tile([D, HB], F32, tag=f"pm{h}")
                nc.tensor.matmul(pm, pmW, xs[:, k, ch], start=True, stop=True)
                # u overwrites the consumed x slot (partitions 0:64)
                nc.scalar.activation(
                    xs[0:D, k, ch], pm, AF.Gelu, bias=tb[:, t : t + 1], scale=1.0
                )
                g1 = psum.tile([2 * D, HB], F32, tag=f"g1{h}")
                nc.tensor.matmul(g1, g1W, xs[:, k, ch], start=True, stop=True)
                hzr = gpool.tile([2 * D, HB], BF, tag=f"hzr{h}")
                nc.scalar.activation(
                    hzr, g1, AF.Tanh, bias=hzr_bias, scale=hzr_scale
                )
                v = gpool.tile([2 * D, HB], BF, tag=f"v{h}")
                nc.gpsimd.tensor_scalar(
                    v[D : 2 * D, :], hzr[D : 2 * D, :], -0.5, -0.5,
                    OP.mult, OP.add,
                )
                g2 = psum.tile([2 * D, HB], F32, tag=f"g2{h}")
                nc.tensor.matmul(g2, g2W, xs[:, k, ch], start=True, stop=False)
                nc.tensor.matmul(
                    g2[0:D, :], prefh, ones_sb, start=False, stop=True
                )
                q = gpool.tile([D, HB], BF, tag=f"q{h}")
                nc.vector.scalar_tensor_tensor(
                    q, hzr[0:D, :], 1.0, g2[0:D, :], OP.add, OP.mult
                )
                hzr_t.append(hzr)
                q_t.append(q)
                g2_t.append(g2)
                v_t.append(v)

            # back phase: w -> nbar -> d -> u2 -> s'
            for h in range(2):
                nc.tensor.matmul(
                    g2_t[h][D : 2 * D, :], ident, q_t[h], start=False, stop=True
                )
            nbar_t = []
            for h in range(2):
                nbar = gpool.tile([2 * D, HB], BF, tag=f"nbar{h}")
                nc.scalar.activation(
                    nbar[D : 2 * D, :], g2_t[h][D : 2 * D, :], AF.Tanh,
                    bias=nb_bias, scale=-1.0,
                )
                nbar_t.append(nbar)
            d_t = []
            for h in range(2):
                d = gpool.tile([2 * D, HB], BF, tag=f"d{h}")
                nc.vector.tensor_add(
                    d[D : 2 * D, :], xs[D : 2 * D, k, CHS[h]],
                    nbar_t[h][D : 2 * D, :],
                )
                d_t.append(d)
            e_t = []
            for h in range(2):
                e = gpool.tile([2 * D, HB], BF, tag=f"e{h}")
                nc.gpsimd.tensor_tensor(
                    e[D : 2 * D, :], v_t[h][D : 2 * D, :],
                    d_t[h][D : 2 * D, :], OP.mult,
                )
                e_t.append(e)
            for h in range(2):
                ch = CHS[h]
                nc.gpsimd.tensor_tensor(
                    xs[D : 2 * D, k + 1, ch], xs[D : 2 * D, k, ch],
                    e_t[h][D : 2 * D, :], OP.add,
                )
                if k == CH - 1 and t + 1 < T:
                    # state carry into the next chunk tile (slot 0)
                    nc.gpsimd.tensor_tensor(
                        xs_next[D : 2 * D, 0, ch], xs[D : 2 * D, k, ch],
                        e_t[h][D : 2 * D, :], OP.add,
                    )

            if k == CH - 1:
                nc.sync.dma_start(
                    out=outT[t - CH + 1 : t + 1, :, :].rearrange("c p r -> p c r"),
                    in_=xs[D : 2 * D, 1 : CH + 1, :],
                )
                xs = xs_next

    nc.compile()
    return nc


def _prep_host(x, mask, msg_W, msg_b, W_ih, W_hh, b_ih, b_hh, basis_freq, phase):
    """Host-side prep: sharding/layout + tiny weight preprocessing."""
    x = np.asarray(x, dtype=np.float32)
    mask = np.asarray(mask)
    msg_W = np.asarray(msg_W, np.float32)
    msg_b = np.asarray(msg_b, np.float32)
    W_ih = np.asarray(W_ih, np.float32)
    W_hh = np.asarray(W_hh, np.float32)
    b_ih = np.asarray(b_ih, np.float32)
    b_hh = np.asarray(b_hh, np.float32)
    basis_freq = np.asarray(basis_freq, np.float32)
    phase = np.asarray(phase, np.float32)

    tr = np.arange(T, dtype=np.int64) * mask.astype(np.int64)
    identity_gather = bool(np.array_equal(tr, np.arange(T)))

    xf = x.reshape(T, B * C, D)
    s0_rows = xf.mean(axis=0)  # [B*C, D] f32 (from ungathered x)
    if not identity_gather:
        xf = xf[tr]

    x4 = xf.reshape(T, NCORES, ROWS, D)
    xT8 = [
        np.ascontiguousarray(x4[:, c].transpose(0, 2, 1)).astype(BF16)
        for c in range(NCORES)
    ]
    s08 = [
        np.ascontiguousarray(s0_rows[c * ROWS : (c + 1) * ROWS].T).astype(BF16)
        for c in range(NCORES)
    ]

    ts_ = np.arange(T, dtype=np.float32)[tr]
    te = np.cos(ts_[:, None] * basis_freq[None, :] + phase[None, :])  # [T, D]
    Wt = msg_W[:, 2 * D : 3 * D]
    tb_host = (te @ Wt.T + msg_b[None, :]).T.astype(np.float32)  # [D, T]

    # torch GRU gate order in W_ih/W_hh: r (0:D), z (D:2D), n (2D:3D)
    wblob = np.zeros((2 * D, 7 * D), np.float32)
    wblob[0:D, 0:D] = msg_W[:, 0:D].T
    wblob[D : 2 * D, 0:D] = msg_W[:, D : 2 * D].T
    # g1W: r cols first (out partitions 0:64), then z (64:128)
    wblob[0:D, D : 2 * D] = W_ih[0:D].T
    wblob[0:D, 2 * D : 3 * D] = W_ih[D : 2 * D].T
    wblob[D : 2 * D, D : 2 * D] = W_hh[0:D].T
    wblob[D : 2 * D, 2 * D : 3 * D] = W_hh[D : 2 * D].T
    # g2W: hh -> out 0:64 (from s), i_n -> out 64:128 (from u)
    wblob[D : 2 * D, 3 * D : 4 * D] = 0.5 * W_hh[2 * D : 3 * D].T
    wblob[0:D, 4 * D : 5 * D] = W_ih[2 * D : 3 * D].T
    wblob[0:D, 5 * D : 6 * D] = np.eye(D, dtype=np.float32)
    wblob[0, 6 * D : 7 * D] = 0.5 * b_hh[2 * D : 3 * D]

    cblob = np.zeros((2 * D, T + 3), np.float32)
    cblob[0:D, 0:T] = tb_host
    cblob[0:D, T] = 0.5
    cblob[D : 2 * D, T] = -0.5
    cblob[0:D, T + 1] = 0.5 * (b_ih[0:D] + b_hh[0:D])
    cblob[D : 2 * D, T + 1] = -0.5 * (b_ih[D : 2 * D] + b_hh[D : 2 * D])
    cblob[D : 2 * D, T + 2] = -b_ih[2 * D : 3 * D]

    shared = {"wblob": wblob.astype(BF16), "cblob": cblob}
    in_maps = []
    for c in range(NCORES):
        m = dict(shared)
        m["xT"] = xT8[c]
        m["s0"] = s08[c]
        in_maps.append(m)
    return in_maps


def kernel(**inputs):
    from concourse.bass_utils import run_bass_kernel_spmd

    in_maps = _prep_host(**inputs)

    if "prog" not in _PROGRAM_CACHE:
        _PROGRAM_CACHE["prog"] = _build_program()
    nc = _PROGRAM_CACHE["prog"]

    res = run_bass_kernel_spmd(nc, in_maps, core_ids=list(range(NCORES)))
    _PROGRAM_CACHE["last_results"] = res

    out = np.empty((T, B * C, D), dtype=np.float32)
    for c in range(NCORES):
        outT_c = np.asarray(res.results[c]["outT"], dtype=np.float32)
        out[:, c * ROWS : (c + 1) * ROWS, :] = outT_c.transpose(0, 2, 1)
    return out.reshape(T, B, C, D)
